# revision 3
# baseline (speedup 1.0000x reference)
"""EMA (exponential moving average) linear recurrence on 8 trn2 NeuronCores.

y[0] = x[0]; y[t] = s*x[t] + (1-s)*y[t-1],  s = 0.3, x: (64, 4096, 256) fp32.

Algorithm: with a = 1-s = 0.7, a^128 ~ 1.6e-20, so history beyond 256 steps is
far below fp32 resolution. Chunk T into blocks of L=128 and write the scan as a
blocked FIR evaluated on the TensorEngine:

    y_c = M @ x_c + P @ x_{c-1}        (chunk 0: y_0 = M0 @ x_0)

with constant 128x128 matrices
    M[i,j]  = s * a^(i-j)   (j <= i),   M0 = M with column 0 scaled to a^i
    P[i,j]  = s * a^(i+128-j)           (dropped terms <= s*a^256 ~ 1e-40)

Sharding: batch B=64 split across the 8 cores (8 rows each); the recurrence is
along T only, so no cross-core communication is needed.

Per core the rhs of each matmul is the chunk tile [128(t) x 2048(b*d)] and the
weights are stationary. The M-part runs in exact fp32 (4 cyc/row); the smaller
cross-chunk correction P runs in float32r (1 cyc/row, ~13 mantissa bits, error
~1e-5 of output scale).
"""
import numpy as np

import concourse.bacc as bacc
import concourse.mybir as mybir
from concourse import tile
from concourse.bass_utils import run_bass_kernel_spmd

S = 0.3
A = 1.0 - S
B, T, D = 64, 4096, 256
NCORES = 8
BC = B // NCORES          # 8 batch rows per core
L = 128                   # chunk length along T == matmul contraction dim
NCH = T // L              # 32 chunks
CB = BC * D               # 2048 free elements per chunk
NSL = CB // 512           # 4 matmul slices (one PSUM bank each)

f32 = mybir.dt.float32
f32r = mybir.dt.float32r

_nc_cache = []


def _weights():
    i = np.arange(L, dtype=np.float64)[:, None]
    j = np.arange(L, dtype=np.float64)[None, :]
    M = np.where(j <= i, S * A ** (i - j), 0.0)
    M0 = M.copy()
    M0[:, 0] = A ** i[:, 0]
    P = S * A ** (i + L - j)
    # lhsT layout [K, M_out] = W.T
    return (
        np.ascontiguousarray(M0.T, dtype=np.float32),
        np.ascontiguousarray(M.T, dtype=np.float32),
        np.ascontiguousarray(P.T, dtype=np.float32),
    )


def _build():
    nc = bacc.Bacc("TRN2", target_bir_lowering=False, debug=False)
    x = nc.dram_tensor("x", [BC, T, D], f32, kind="ExternalInput").ap()
    wm0 = nc.dram_tensor("wm0", [L, L], f32, kind="ExternalInput").ap()
    wm = nc.dram_tensor("wm", [L, L], f32, kind="ExternalInput").ap()
    wp = nc.dram_tensor("wp", [L, L], f32r, kind="ExternalInput").ap()
    y = nc.dram_tensor("y", [BC, T, D], f32, kind="ExternalOutput").ap()

    with tile.TileContext(nc) as tc, \
         tc.tile_pool(name="w", bufs=1) as wpool, \
         tc.tile_pool(name="xs", bufs=6) as xpool, \
         tc.tile_pool(name="xr", bufs=4) as xrpool, \
         tc.tile_pool(name="ys", bufs=4) as ypool, \
         tc.tile_pool(name="ps", bufs=2, space="PSUM") as pspool:
        wm0_t = wpool.tile([L, L], f32)
        nc.sync.dma_start(wm0_t[:], wm0[:])
        wm_t = wpool.tile([L, L], f32)
        nc.sync.dma_start(wm_t[:], wm[:])
        wp_t = wpool.tile([L, L], f32r)
        nc.sync.dma_start(wp_t[:], wp[:])

        prev_xr = None
        for c in range(NCH):
            xt = xpool.tile([L, CB], f32)
            # DRAM view [p(t), b, d]: 3D AP, 1 KiB contiguous runs
            src = x[:, c * L:(c + 1) * L, :].rearrange("b p d -> p b d")
            nc.sync.dma_start(xt[:].rearrange("p (b d) -> p b d", b=BC), src)

            ps = pspool.tile([L, CB], f32)
            wmt = wm0_t if c == 0 else wm_t
            for n in range(NSL):
                nc.tensor.matmul(
                    ps[:, n * 512:(n + 1) * 512],
                    wmt[:],
                    xt[:, n * 512:(n + 1) * 512],
                    start=True,
                    stop=(c == 0),
                )
            if c > 0:
                for n in range(NSL):
                    nc.tensor.matmul(
                        ps[:, n * 512:(n + 1) * 512],
                        wp_t[:],
                        prev_xr[:, n * 512:(n + 1) * 512],
                        start=False,
                        stop=True,
                    )
            # f32r-rounded copy of this chunk, feeding chunk c+1's P matmul.
            # Alternate DVE/ACT against the evacuation to balance engines.
            if c < NCH - 1:
                xr = xrpool.tile([L, CB], f32r)
                if c % 2 == 0:
                    nc.vector.tensor_copy(xr[:], xt[:])
                else:
                    nc.scalar.copy(xr[:], xt[:])
                prev_xr = xr
            yt = ypool.tile([L, CB], f32)
            if c % 2 == 0:
                nc.scalar.copy(yt[:], ps[:])
            else:
                nc.vector.tensor_copy(yt[:], ps[:])
            dst = y[:, c * L:(c + 1) * L, :].rearrange("b p d -> p b d")
            nc.sync.dma_start(dst, yt[:].rearrange("p (b d) -> p b d", b=BC))
    nc.compile()
    return nc


def get_nc():
    if not _nc_cache:
        _nc_cache.append(_build())
    return _nc_cache[0]


def make_in_maps(x: np.ndarray):
    x = np.ascontiguousarray(np.asarray(x), dtype=np.float32)
    assert x.shape == (B, T, D)
    wm0, wm, wp = _weights()
    return [
        {"x": x[i * BC:(i + 1) * BC], "wm0": wm0, "wm": wm, "wp": wp}
        for i in range(NCORES)
    ]


def kernel(x: np.ndarray) -> np.ndarray:
    res = run_bass_kernel_spmd(
        get_nc(), make_in_maps(x), list(range(NCORES))
    ).results
    return np.concatenate([res[i]["y"] for i in range(NCORES)], axis=0)


# revision 5
# speedup vs baseline: 1.1041x; 1.1041x over previous
"""EMA (exponential moving average) linear recurrence on 8 trn2 NeuronCores.

y[0] = x[0]; y[t] = s*x[t] + (1-s)*y[t-1],  s = 0.3, x: (64, 4096, 256) fp32.

Algorithm: with a = 1-s = 0.7, a^128 ~ 1.6e-20, so history beyond 256 steps is
far below fp32 resolution. Chunk T into blocks of L=128 and write the scan as a
blocked FIR evaluated on the TensorEngine:

    y_c = M @ x_c + P @ x_{c-1}        (chunk 0: y_0 = M0 @ x_0)

with constant 128x128 matrices
    M[i,j]  = s * a^(i-j)   (j <= i),   M0 = M with column 0 scaled to a^i
    P[i,j]  = s * a^(i+128-j)           (dropped terms <= s*a^256 ~ 1e-40)

Sharding: batch B=64 split across the 8 cores (8 rows each); the recurrence is
along T only, so no cross-core communication is needed.

Precision: each matmul runs in fp16 hi/lo split form (1 cyc/row on the PE vs 4
for fp32, and fp16 weights get fast-weight-load). With x = xh + xl and
W = Wh + Wl (fp16 rounding residues), W@x ~ Wh@xh + Wh@xl + Wl@xh; the dropped
Wl@xl term is ~2^-22 relative, giving fp32-quality results (absmax ~8e-7 vs
the jax reference) at 6 passes/chunk of PE streaming. The xh/xl split is done
on ACT (cast copy) and DVE (subtract), which are otherwise idle; the kernel
stays DMA-bound (~64 MiB/core at ~358 GB/s).
"""
import numpy as np

import concourse.bacc as bacc
import concourse.mybir as mybir
from concourse import tile
from concourse.bass_utils import run_bass_kernel_spmd

S = 0.3
A = 1.0 - S
B, T, D = 64, 4096, 256
NCORES = 8
BC = B // NCORES          # 8 batch rows per core
L = 128                   # chunk length along T == matmul contraction dim
NCH = T // L              # 32 chunks
CB = BC * D               # 2048 free elements per chunk
NSL = CB // 512           # 4 matmul slices (one PSUM bank each)

f32 = mybir.dt.float32
f16 = mybir.dt.float16

_nc_cache = []


def _weights():
    i = np.arange(L, dtype=np.float64)[:, None]
    j = np.arange(L, dtype=np.float64)[None, :]
    M = np.where(j <= i, S * A ** (i - j), 0.0)
    M0 = M.copy()
    M0[:, 0] = A ** i[:, 0]
    P = S * A ** (i + L - j)

    def split(w):
        # lhsT layout [K, M_out] = W.T; fp16 hi + residue lo
        wT = w.T
        wh = wT.astype(np.float16)
        wl = (wT - wh.astype(np.float64)).astype(np.float16)
        return np.ascontiguousarray(wh), np.ascontiguousarray(wl)

    return split(M0), split(M), split(P)


def _build():
    nc = bacc.Bacc("TRN2", target_bir_lowering=False, debug=False)
    x = nc.dram_tensor("x", [BC, T, D], f32, kind="ExternalInput").ap()
    wnames = ("wm0h", "wm0l", "wmh", "wml", "wph", "wpl")
    wd = {
        n: nc.dram_tensor(n, [L, L], f16, kind="ExternalInput").ap()
        for n in wnames
    }
    y = nc.dram_tensor("y", [BC, T, D], f32, kind="ExternalOutput").ap()

    with tile.TileContext(nc) as tc, \
         tc.tile_pool(name="w", bufs=1) as wpool, \
         tc.tile_pool(name="xs", bufs=5) as xpool, \
         tc.tile_pool(name="xh", bufs=4) as xhpool, \
         tc.tile_pool(name="xl", bufs=4) as xlpool, \
         tc.tile_pool(name="ys", bufs=4) as ypool, \
         tc.tile_pool(name="ps", bufs=2, space="PSUM") as pspool:
        wt = {}
        for n in wnames:
            wt[n] = wpool.tile([L, L], f16, tag=n, name=n)
            nc.sync.dma_start(wt[n][:], wd[n][:])

        prev_xh = prev_xl = None
        for c in range(NCH):
            xt = xpool.tile([L, CB], f32)
            # DRAM view [p(t), b, d]: 3D AP, 1 KiB contiguous runs
            src = x[:, c * L:(c + 1) * L, :].rearrange("b p d -> p b d")
            nc.sync.dma_start(xt[:].rearrange("p (b d) -> p b d", b=BC), src)

            xh = xhpool.tile([L, CB], f16)
            nc.scalar.copy(xh[:], xt[:])            # ACT: hi = fp16(x)
            xl = xlpool.tile([L, CB], f16)
            nc.vector.tensor_sub(xl[:], xt[:], xh[:])  # DVE: lo = x - hi

            ps = pspool.tile([L, CB], f32)
            mh = wt["wm0h"] if c == 0 else wt["wmh"]
            ml = wt["wm0l"] if c == 0 else wt["wml"]
            # grouped by stationary weight to allow weight-load reuse
            for rhs in (xh, xl):
                for n in range(NSL):
                    nc.tensor.matmul(
                        ps[:, n * 512:(n + 1) * 512], mh[:],
                        rhs[:, n * 512:(n + 1) * 512],
                        start=(rhs is xh), stop=False,
                    )
            for n in range(NSL):
                nc.tensor.matmul(
                    ps[:, n * 512:(n + 1) * 512], ml[:],
                    xh[:, n * 512:(n + 1) * 512],
                    start=False, stop=(c == 0),
                )
            if c > 0:
                for rhs in (prev_xh, prev_xl):
                    for n in range(NSL):
                        nc.tensor.matmul(
                            ps[:, n * 512:(n + 1) * 512], wt["wph"][:],
                            rhs[:, n * 512:(n + 1) * 512],
                            start=False, stop=False,
                        )
                for n in range(NSL):
                    nc.tensor.matmul(
                        ps[:, n * 512:(n + 1) * 512], wt["wpl"][:],
                        prev_xh[:, n * 512:(n + 1) * 512],
                        start=False, stop=True,
                    )

            yt = ypool.tile([L, CB], f32)
            if c % 2 == 0:
                nc.scalar.copy(yt[:], ps[:])
            else:
                nc.vector.tensor_copy(yt[:], ps[:])
            dst = y[:, c * L:(c + 1) * L, :].rearrange("b p d -> p b d")
            nc.scalar.dma_start(dst, yt[:].rearrange("p (b d) -> p b d", b=BC))
            prev_xh, prev_xl = xh, xl
    nc.compile()
    return nc


def get_nc():
    if not _nc_cache:
        _nc_cache.append(_build())
    return _nc_cache[0]


def make_in_maps(x: np.ndarray):
    x = np.ascontiguousarray(np.asarray(x), dtype=np.float32)
    assert x.shape == (B, T, D)
    (wm0h, wm0l), (wmh, wml), (wph, wpl) = _weights()
    w = {"wm0h": wm0h, "wm0l": wm0l, "wmh": wmh,
         "wml": wml, "wph": wph, "wpl": wpl}
    return [{"x": x[i * BC:(i + 1) * BC], **w} for i in range(NCORES)]


def kernel(x: np.ndarray) -> np.ndarray:
    res = run_bass_kernel_spmd(
        get_nc(), make_in_maps(x), list(range(NCORES))
    ).results
    return np.concatenate([res[i]["y"] for i in range(NCORES)], axis=0)


# revision 9
# speedup vs baseline: 1.1098x; 1.0052x over previous
"""EMA (exponential moving average) linear recurrence on 8 trn2 NeuronCores.

y[0] = x[0]; y[t] = s*x[t] + (1-s)*y[t-1],  s = 0.3, x: (64, 4096, 256) fp32.

Algorithm: with a = 1-s = 0.7, a^128 ~ 1.6e-20, so history beyond 256 steps is
far below fp32 resolution. Chunk T into blocks of L=128 and write the scan as a
blocked FIR evaluated on the TensorEngine:

    y_c = M @ x_c + P @ x_{c-1}        (chunk 0: y_0 = M0 @ x_0)

with constant 128x128 matrices
    M[i,j]  = s * a^(i-j)   (j <= i),   M0 = M with column 0 scaled to a^i
    P[i,j]  = s * a^(i+128-j)           (dropped terms <= s*a^256 ~ 1e-40)

Sharding: batch B=64 split across the 8 cores (8 rows each); the recurrence is
along T only, so no cross-core communication is needed.

Precision: each matmul runs in fp16 hi/lo split form (1 cyc/row on the PE vs 4
for fp32, and fp16 weights get fast-weight-load). With x = xh + xl and
W = Wh + Wl (fp16 rounding residues), W@x ~ Wh@xh + Wh@xl + Wl@xh; the dropped
Wl@xl term is ~2^-22 relative, giving fp32-quality results (absmax ~8e-7 vs
the jax reference) at 6 passes/chunk of PE streaming. The xh/xl split is done
on ACT (cast copy) and DVE (subtract), which are otherwise idle; the kernel
stays DMA-bound (~64 MiB/core at ~358 GB/s).
"""
import numpy as np

import concourse.bacc as bacc
import concourse.mybir as mybir
from concourse import tile
from concourse.bass_utils import run_bass_kernel_spmd

S = 0.3
A = 1.0 - S
B, T, D = 64, 4096, 256
NCORES = 8
BC = B // NCORES          # 8 batch rows per core
L = 128                   # chunk length along T == matmul contraction dim
NCH = T // L              # 32 chunks
CB = BC * D               # 2048 free elements per chunk
NSL = CB // 512           # 4 matmul slices (one PSUM bank each)

f32 = mybir.dt.float32
f16 = mybir.dt.float16

_nc_cache = []


def _weights():
    i = np.arange(L, dtype=np.float64)[:, None]
    j = np.arange(L, dtype=np.float64)[None, :]
    M = np.where(j <= i, S * A ** (i - j), 0.0)
    M0 = M.copy()
    M0[:, 0] = A ** i[:, 0]
    P = S * A ** (i + L - j)

    def split(w):
        # lhsT layout [K, M_out] = W.T; fp16 hi + residue lo
        wT = w.T
        wh = wT.astype(np.float16)
        wl = (wT - wh.astype(np.float64)).astype(np.float16)
        return np.ascontiguousarray(wh), np.ascontiguousarray(wl)

    return split(M0), split(M), split(P)


def _build():
    nc = bacc.Bacc("TRN2", target_bir_lowering=False, debug=False)
    x = nc.dram_tensor("x", [BC, T, D], f32, kind="ExternalInput").ap()
    wnames = ("wm0h", "wm0l", "wmh", "wml", "wph", "wpl")
    # all six weight matrices in one tensor -> one DMA at kernel start
    wall = nc.dram_tensor("wall", [L, 6 * L], f16, kind="ExternalInput").ap()
    y = nc.dram_tensor("y", [BC, T, D], f32, kind="ExternalOutput").ap()

    with tile.TileContext(nc) as tc, \
         tc.tile_pool(name="w", bufs=1) as wpool, \
         tc.tile_pool(name="xs", bufs=5) as xpool, \
         tc.tile_pool(name="xh", bufs=4) as xhpool, \
         tc.tile_pool(name="xl", bufs=4) as xlpool, \
         tc.tile_pool(name="ys", bufs=4) as ypool, \
         tc.tile_pool(name="ps", bufs=2, space="PSUM") as pspool:
        wall_t = wpool.tile([L, 6 * L], f16)
        # SWDGE ring: keeps the sync/scalar HWDGE rings free for x/y traffic
        nc.gpsimd.dma_start(wall_t[:], wall[:])
        wt = {n: wall_t[:, k * L:(k + 1) * L] for k, n in enumerate(wnames)}

        prev_xh = prev_xl = None
        for c in range(NCH):
            xt = xpool.tile([L, CB], f32)
            # DRAM view [p(t), b, d]: 3D AP, 1 KiB contiguous runs
            src = x[:, c * L:(c + 1) * L, :].rearrange("b p d -> p b d")
            nc.sync.dma_start(xt[:].rearrange("p (b d) -> p b d", b=BC), src)

            xh = xhpool.tile([L, CB], f16)
            nc.scalar.copy(xh[:], xt[:])            # ACT: hi = fp16(x)
            xl = xlpool.tile([L, CB], f16)
            nc.vector.tensor_sub(xl[:], xt[:], xh[:])  # DVE: lo = x - hi

            ps = pspool.tile([L, CB], f32)
            mh = wt["wm0h"] if c == 0 else wt["wmh"]
            ml = wt["wm0l"] if c == 0 else wt["wml"]
            # grouped by stationary weight to allow weight-load reuse
            for rhs in (xh, xl):
                for n in range(NSL):
                    nc.tensor.matmul(
                        ps[:, n * 512:(n + 1) * 512], mh,
                        rhs[:, n * 512:(n + 1) * 512],
                        start=(rhs is xh), stop=False,
                    )
            for n in range(NSL):
                nc.tensor.matmul(
                    ps[:, n * 512:(n + 1) * 512], ml,
                    xh[:, n * 512:(n + 1) * 512],
                    start=False, stop=(c == 0),
                )
            if c > 0:
                for rhs in (prev_xh, prev_xl):
                    for n in range(NSL):
                        nc.tensor.matmul(
                            ps[:, n * 512:(n + 1) * 512], wt["wph"],
                            rhs[:, n * 512:(n + 1) * 512],
                            start=False, stop=False,
                        )
                for n in range(NSL):
                    nc.tensor.matmul(
                        ps[:, n * 512:(n + 1) * 512], wt["wpl"],
                        prev_xh[:, n * 512:(n + 1) * 512],
                        start=False, stop=True,
                    )

            yt = ypool.tile([L, CB], f32)
            if c % 2 == 0:
                nc.scalar.copy(yt[:], ps[:])
            else:
                nc.vector.tensor_copy(yt[:], ps[:])
            dst = y[:, c * L:(c + 1) * L, :].rearrange("b p d -> p b d")
            nc.scalar.dma_start(dst, yt[:].rearrange("p (b d) -> p b d", b=BC))
            prev_xh, prev_xl = xh, xl
    nc.compile()
    return nc


def get_nc():
    if not _nc_cache:
        _nc_cache.append(_build())
    return _nc_cache[0]


def make_in_maps(x: np.ndarray):
    x = np.ascontiguousarray(np.asarray(x), dtype=np.float32)
    assert x.shape == (B, T, D)
    (wm0h, wm0l), (wmh, wml), (wph, wpl) = _weights()
    wall = np.ascontiguousarray(
        np.concatenate([wm0h, wm0l, wmh, wml, wph, wpl], axis=1)
    )
    return [{"x": x[i * BC:(i + 1) * BC], "wall": wall} for i in range(NCORES)]


def kernel(x: np.ndarray) -> np.ndarray:
    res = run_bass_kernel_spmd(
        get_nc(), make_in_maps(x), list(range(NCORES))
    ).results
    return np.concatenate([res[i]["y"] for i in range(NCORES)], axis=0)
